# revision 41
# baseline (speedup 1.0000x reference)
"""Multi-head attention (B=2, S=2048, D=1024, H=16, DH=64) on 8 TRN2 cores.

Sharding: core c handles batch b = c//4 and head group g = c%4 (4 heads).
Per core, for its (b, g):
    VhT/KhT/QhT = per-head projections in transposed layout [e, s],
    Vh = PE-transposed back to [j, e] with a ones column appended (vhe),
    S^T = Kh @ Qh^T per head (scores transposed, keys j on partitions),
    P^T = exp(S^T / sqrt(dk))  (no max subtraction; fp32 range is ample),
    acc = Vh_ext^T @ P^T  (row 64 = softmax denominators via the ones col),
    outT = acc[0:64] * (1/l) broadcast  (PE ones-outer-product broadcast),
    PT_partial = sum_c Wf[c,:] outT[c,:]  -> partial final projection [D, S].
Host: out[b] = (sum_g PT_partial).T + bf.

Schedule: V and K stream first (full S), then Q streams in two i-halves;
attention + the final projection for each i-half overlap the later streams.

Data path is bf16: streams, weights, P (exp output), V tiles, and the
final output partials are all bf16 (halves DMA traffic; matmul rate on
TRN2 is the same 1 row/cycle for bf16 as float32r). PSUM accumulation
stays fp32.
"""

import sys

sys.path.insert(0, "/opt/trn_rl_repo")

from contextlib import ExitStack

import ml_dtypes
import numpy as np

import concourse.mybir as mybir
import concourse.tile as tile
from concourse import bacc
from concourse.bass_utils import run_bass_kernel_spmd

B, S, D, H, DH = 2, 2048, 1024, 16, 64
NCORES = 8
GPB = 4  # head-group cores per batch
HPG = H // GPB  # heads per group (4)
CW = HPG * DH  # concat width per core (256)
NPAIR = HPG // 2  # head pairs per group (2)
DCH = D // 128  # d chunks (8)
JCH = S // 128  # key chunks (16)
IB = 1024  # i-block width for attention
NIB = S // IB  # 2
F32 = mybir.dt.float32
BF16 = mybir.dt.bfloat16
FP8 = mybir.dt.float8e4
AF = mybir.ActivationFunctionType
INV_SQRT_DK = 1.0 / np.sqrt(DH)
BFNP = ml_dtypes.bfloat16

_CACHE = {}


def _build():
    nc = bacc.Bacc("TRN2", target_bir_lowering=False, debug=False, num_devices=NCORES)

    qt_d = nc.dram_tensor("qt", [D, S], BF16, kind="ExternalInput").ap()
    kt_d = nc.dram_tensor("kt", [D, S], BF16, kind="ExternalInput").ap()
    vt_d = nc.dram_tensor("vt", [D, S], BF16, kind="ExternalInput").ap()
    wq_d = nc.dram_tensor("wq", [D, CW], BF16, kind="ExternalInput").ap()
    wk_d = nc.dram_tensor("wk", [D, CW], BF16, kind="ExternalInput").ap()
    wv_d = nc.dram_tensor("wv", [D, CW], BF16, kind="ExternalInput").ap()
    wf_d = nc.dram_tensor("wf", [CW, D], BF16, kind="ExternalInput").ap()
    bq_d = nc.dram_tensor("bq", [CW], F32, kind="ExternalInput").ap()
    bk_d = nc.dram_tensor("bk", [CW], F32, kind="ExternalInput").ap()
    bv_d = nc.dram_tensor("bv", [1, CW], BF16, kind="ExternalInput").ap()
    ones_d = nc.dram_tensor("ones32", [128, 2 * JCH, 1], BF16, kind="ExternalInput").ap()
    onesr_d = nc.dram_tensor("ones_row", [1, 128], BF16, kind="ExternalInput").ap()
    pt_d = nc.dram_tensor("pt", [D, S], BF16, kind="ExternalOutput").ap()

    with (
        tile.TileContext(nc) as tc,
        nc.allow_low_precision(reason="bf16 data path is intentional"),
        ExitStack() as ctx,
    ):
        const = ctx.enter_context(tc.tile_pool(name="const", bufs=1))
        persist = ctx.enter_context(tc.tile_pool(name="persist", bufs=1))

        wq_sb = const.tile([128, DCH * CW], BF16, tag="wq")
        wk_sb = const.tile([128, DCH * CW], BF16, tag="wk")
        wv_sb = const.tile([128, DCH * CW], BF16, tag="wv")
        wf_sb = const.tile([128, 2 * D], BF16, tag="wf")
        bq_sb = const.tile([128, NPAIR], F32, tag="bq")
        bk_sb = const.tile([128, NPAIR], F32, tag="bk")
        bv_sb = const.tile([1, CW], BF16, tag="bv")
        ones128 = const.tile([1, 128], BF16, tag="ones")
        ones32 = const.tile([128, 2 * JCH, 1], BF16, tag="ones32")

        def load_w(w_sb, w_dram):
            nc.sync.dma_start(
                out=w_sb[:].rearrange("p (c e) -> p c e", c=DCH),
                in_=w_dram.rearrange("(c p) e -> p c e", p=128),
            )

        def load_b(b_sb, b_dram):
            nc.sync.dma_start(out=b_sb[:], in_=b_dram.rearrange("(r p) -> p r", p=128))

        qhT = [persist.tile([128, S], BF16, tag=f"qhT{r}", name=f"qhT{r}") for r in range(NPAIR)]
        outT = [persist.tile([128, S], BF16, tag=f"outT{r}", name=f"outT{r}") for r in range(NPAIR)]
        khT = [persist.tile([128, S], BF16, tag=f"khT{r}", name=f"khT{r}") for r in range(NPAIR)]
        vhe = [persist.tile([128, JCH * 130], BF16, tag=f"vhe{r}", name=f"vhe{r}") for r in range(NPAIR)]
        # fp8 hi/lo DoubleRow score operands for the LATE head-blocks only
        # (quantized in the background from the bf16 qhT/khT; head 0/1 keep
        # the bf16 score path so the startup chain is untouched). For head h:
        # mq[h] moving: parts 0:64 = fp8(qhT), 64:128 = fp8 residual, slabs
        # duplicated; stK[h] stationary: slab0 = fp8(khT), slab1 = residual,
        # partition halves duplicated. One DoubleRow matmul contracts
        # (qhi+qlo)(khi+klo) exactly at 0.5 cycles/row.
        mq = [persist.tile([128, 2, S], FP8, tag=f"mq{h}", name=f"mq{h}") for h in range(HPG)]
        stK = [persist.tile([128, 2, S], FP8, tag=f"stK{h}", name=f"stK{h}") for h in range(HPG)]

        # ============ V: project directly into natural [j, e] layout ============
        with tc.tile_pool(name="xt", bufs=8) as xt_pool:
            load_w(wv_sb, wv_d)
            nc.sync.dma_start(out=bv_sb[:], in_=bv_d)
            nc.sync.dma_start(out=ones128[:], in_=onesr_d)
            nc.sync.dma_start(out=ones32[:], in_=ones_d)
            xtv = []
            for d in range(DCH):
                t = xt_pool.tile([128, S], BF16, tag="xt", name="xt_v")
                nc.sync.dma_start(out=t[:], in_=vt_d[128 * d : 128 * (d + 1), :])
                xtv.append(t)
            with tc.tile_pool(name="ps_vh", bufs=8, space="PSUM") as ps_vh_pool, nc.named_scope("vproj"):
                ps_vh = [
                    ps_vh_pool.tile([128, 512], F32, tag="vh", name=f"ps_vh{jb}")
                    for jb in range(JCH // 2)
                ]
                for r in range(NPAIR):
                    nc.vector.tensor_copy(
                        vhe[r][:].rearrange("p (c w) -> p c w", w=65)[:, :, 64:65],
                        ones32[:],
                    )
                for jh in range(2):
                    for jb in range(JCH // 2):
                        j = 2 * jb + jh
                        reg = ps_vh[jb][:, 256 * jh : 256 * (jh + 1)]
                        for d in range(DCH):
                            nc.tensor.matmul(
                                reg,
                                xtv[d][:, 128 * j : 128 * (j + 1)],
                                wv_sb[:, CW * d : CW * (d + 1)],
                                start=(d == 0),
                                stop=False,
                            )
                        nc.tensor.matmul(
                            reg, ones128[:], bv_sb[:], start=False, stop=True
                        )
                        for r in range(NPAIR):
                            dst = vhe[r][:, 130 * j : 130 * j + 130]
                            nc.vector.tensor_copy(
                                dst.rearrange("p (b e) -> p b e", e=65)[:, :, 0:64],
                                reg[:, 128 * r : 128 * (r + 1)]
                                .rearrange("p (b e) -> p b e", e=64),
                            )

            load_w(wq_sb, wq_d)
            load_b(bq_sb, bq_d)
            load_w(wk_sb, wk_d)
            load_b(bk_sb, bk_d)

        # ========= attention-era pools open here =========
        if True:
            with (
                tc.tile_pool(name="qx", bufs=8) as qx_pool,
                tc.tile_pool(name="kx", bufs=10) as kx_pool,
                tc.tile_pool(name="pexp", bufs=6) as pexp_pool,
                tc.tile_pool(name="bc", bufs=2) as bc_pool,
                tc.tile_pool(name="rc", bufs=2) as rc_pool,
                tc.tile_pool(name="fo", bufs=4) as fo_pool,
                tc.tile_pool(name="hl8q", bufs=2) as hl8q_pool,
                tc.tile_pool(name="hl8k", bufs=2) as hl8k_pool,
                tc.tile_pool(name="ps_sc", bufs=2, space="PSUM") as ps_sc,
                tc.tile_pool(name="ps_acc", bufs=2, space="PSUM") as ps_acc,
            ):
                def emit_quantQ(ib_, r):
                    """Background fp8 hi/lo split of qhT[r] cols ib_ into
                    mq[2r], mq[2r+1] (DVE quantize + HWDGE dup; both idle
                    when this runs)."""
                    isl_ = slice(IB * ib_, IB * (ib_ + 1))
                    hl = hl8q_pool.tile([128, 2, IB], FP8, tag="hl8q", name="hl8q")
                    nc.vector.tensor_copy(hl[:, 0, :], qhT[r][:, isl_])
                    nc.vector.tensor_tensor(
                        hl[:, 1, :], qhT[r][:, isl_], hl[:, 0, :],
                        mybir.AluOpType.subtract,
                    )
                    for q in range(2):
                        h = 2 * r + q
                        ph = slice(64 * q, 64 * (q + 1))
                        nc.sync.dma_start(
                            out=mq[h][0:64, :, isl_],
                            in_=hl[ph, 0:1, :].broadcast_to([64, 2, IB]),
                        )
                        nc.sync.dma_start(
                            out=mq[h][64:128, :, isl_],
                            in_=hl[ph, 1:2, :].broadcast_to([64, 2, IB]),
                        )

                def emit_quantK(r):
                    """Background fp8 hi/lo split of the full khT[r] into
                    stK[2r], stK[2r+1] (gpsimd quantize, idle mid-run)."""
                    hl = hl8k_pool.tile([128, 2, S], FP8, tag="hl8k", name="hl8k")
                    nc.gpsimd.tensor_copy(hl[:, 0, :], khT[r][:])
                    nc.gpsimd.tensor_tensor(
                        hl[:, 1, :], khT[r][:], hl[:, 0, :],
                        mybir.AluOpType.subtract,
                    )
                    for q in range(2):
                        h = 2 * r + q
                        ph = slice(64 * q, 64 * (q + 1))
                        nc.sync.dma_start(out=stK[h][0:64, :, :], in_=hl[ph, :, :])
                        nc.sync.dma_start(out=stK[h][64:128, :, :], in_=hl[ph, :, :])

                def emit_scores_fp8(h, s_ps, jc, ib_):
                    k_st = stK[h][:, :, 128 * jc : 128 * (jc + 1)]
                    for k in range(IB // 512):
                        c0 = IB * ib_ + 512 * k
                        nc.tensor.matmul(
                            s_ps[:, 512 * k : 512 * (k + 1)],
                            k_st,
                            mq[h][:, :, c0 : c0 + 512],
                            start=True,
                            stop=True,
                            perf_mode=mybir.MatmulPerfMode.DoubleRow,
                        )

                def emit_q_dmas(ib_):
                    isl_ = slice(IB * ib_, IB * (ib_ + 1))
                    qx = []
                    for d in range(DCH):
                        t = qx_pool.tile([128, IB], BF16, tag="qx", name="qx")
                        nc.sync.dma_start(out=t[:], in_=qt_d[128 * d : 128 * (d + 1), isl_])
                        qx.append(t)
                    return qx

                def make_qproj_closures(ib_, r, qx):
                    isl_ = slice(IB * ib_, IB * (ib_ + 1))
                    state = {}

                    def step(d):
                        if d == 0:
                            state["ps"] = ps_acc.tile([128, IB], F32, tag="acc", name="ps_qd")
                        ps_q = state["ps"]
                        w_st = wq_sb[:, CW * d + 128 * r : CW * d + 128 * (r + 1)]
                        for k in range(IB // 512):
                            nc.tensor.matmul(
                                ps_q[:, 512 * k : 512 * (k + 1)],
                                w_st,
                                qx[d][:, 512 * k : 512 * (k + 1)],
                                start=(d == 0),
                                stop=(d == DCH - 1),
                            )

                    def bias():
                        nc.vector.tensor_scalar_add(
                            qhT[r][:, isl_], state["ps"][:], bq_sb[:, r : r + 1]
                        )

                    return [
                        (lambda d=d: step(d)) for d in range(DCH)
                    ] + [bias, lambda: emit_quantQ(ib_, r)]

                def emit_qproj_pair(ib_, r, qx):
                    with nc.named_scope(f"qproj{ib_}r{r}"):
                        isl_ = slice(IB * ib_, IB * (ib_ + 1))
                        ps_q = ps_sc.tile([128, IB], F32, tag="sc", name="ps_q")
                        for d in range(DCH):
                            w_st = wq_sb[:, CW * d + 128 * r : CW * d + 128 * (r + 1)]
                            for k in range(IB // 512):
                                nc.tensor.matmul(
                                    ps_q[:, 512 * k : 512 * (k + 1)],
                                    w_st,
                                    qx[d][:, 512 * k : 512 * (k + 1)],
                                    start=(d == 0),
                                    stop=(d == DCH - 1),
                                )
                        nc.vector.tensor_scalar_add(
                            qhT[r][:, isl_], ps_q[:], bq_sb[:, r : r + 1]
                        )

                # Q0 stream + projection; fp8 quantize runs in background
                qx0 = emit_q_dmas(0)
                emit_qproj_pair(0, 0, qx0)
                emit_qproj_pair(0, 1, qx0)
                emit_quantQ(0, 0)
                emit_quantQ(0, 1)


                def emit_kproj_pair(sblk, r, kx):
                    ps_kb = ps_sc.tile([128, 512], F32, tag="sc", name="ps_kb")
                    for d in range(DCH):
                        w_st = wk_sb[:, CW * d + 128 * r : CW * d + 128 * (r + 1)]
                        nc.tensor.matmul(
                            ps_kb[:],
                            w_st,
                            kx[d][:],
                            start=(d == 0),
                            stop=(d == DCH - 1),
                        )
                    nc.vector.tensor_scalar_add(
                        khT[r][:, 512 * sblk : 512 * (sblk + 1)],
                        ps_kb[:],
                        bk_sb[:, r : r + 1],
                    )

                def emit_final_tile(ib_, f, i4, pool=None, copy_eng=None):
                    i0 = IB * ib_ + 512 * i4
                    pf = (pool or ps_sc).tile(
                        [128, 512], F32, tag="sc" if pool is None else "acc", name="pf"
                    )
                    for cc in range(2):
                        nc.tensor.matmul(
                            pf[:],
                            wf_sb[:, D * cc + 128 * f : D * cc + 128 * (f + 1)],
                            outT[cc][:, i0 : i0 + 512],
                            start=(cc == 0),
                            stop=(cc == 1),
                        )
                    fo = fo_pool.tile([128, 512], BF16, tag="fo", name="fo")
                    if copy_eng == "act":
                        nc.scalar.copy(fo[:], pf[:])
                    else:
                        nc.vector.tensor_copy(fo[:], pf[:])
                    nc.sync.dma_start(
                        out=pt_d[128 * f : 128 * (f + 1), i0 : i0 + 512],
                        in_=fo[:],
                    )

                def emit_norm(pend):
                    acc_, r_, qs_, isl_ = pend
                    with nc.named_scope("norm"):
                        rc = rc_pool.tile([1, IB], BF16, tag="rc", name="rc")
                        nc.vector.reciprocal(rc[:], acc_[64:65, :])
                        bc_ps = ps_sc.tile([128, IB], F32, tag="sc", name="bc_ps")
                        for k in range(IB // 512):
                            nc.tensor.matmul(
                                bc_ps[0:64, 512 * k : 512 * (k + 1)],
                                ones128[0:1, 0:64],
                                rc[:, 512 * k : 512 * (k + 1)],
                                start=True,
                                stop=True,
                            )
                        bc_sb = bc_pool.tile([64, IB], F32, tag="bc", name="bc_sb")
                        nc.vector.tensor_copy(bc_sb[:], bc_ps[0:64, :])
                        nc.vector.tensor_tensor(
                            outT[r_][qs_, isl_],
                            acc_[0:64, :],
                            bc_sb[:],
                            mybir.AluOpType.mult,
                        )

                pending_norm_box = [None]

                def emit_attention(ib_, deferred, heads=range(HPG), flush_norm=True, fp8=()):
                    isl_ = slice(IB * ib_, IB * (ib_ + 1))
                    pending_norm = pending_norm_box[0]
                    for h in heads:
                        with nc.named_scope(f"attn{ib_}h{h}"):
                            r, q = h // 2, h % 2
                            qs = slice(64 * q, 64 * (q + 1))
                            acc = ps_acc.tile([65, IB], F32, tag="acc", name="acc")
                            prev_pv = None
                            for jc in range(JCH):
                                s_ps = ps_sc.tile([128, IB], F32, tag="sc", name="s_ps")
                                if h in fp8:
                                    emit_scores_fp8(h, s_ps, jc, ib_)
                                else:
                                    k_st = khT[r][qs, 128 * jc : 128 * (jc + 1)]
                                    for k in range(IB // 512):
                                        nc.tensor.matmul(
                                            s_ps[:, 512 * k : 512 * (k + 1)],
                                            k_st,
                                            qhT[r][qs, IB * ib_ + 512 * k : IB * ib_ + 512 * (k + 1)],
                                            start=True,
                                            stop=True,
                                        )
                                if prev_pv is not None:
                                    v_st_p, pexp_p, jc_p = prev_pv
                                    for k in range(IB // 512):
                                        nc.tensor.matmul(
                                            acc[:, 512 * k : 512 * (k + 1)],
                                            v_st_p,
                                            pexp_p[:, 512 * k : 512 * (k + 1)],
                                            start=(jc_p == 0),
                                            stop=False,
                                        )
                                pexp = pexp_pool.tile([128, IB], BF16, tag="pexp", name="pexp")
                                nc.scalar.activation(pexp[:], s_ps[:], AF.Exp, scale=INV_SQRT_DK)
                                prev_pv = (
                                    vhe[r][:, 130 * jc + 65 * q : 130 * jc + 65 * (q + 1)],
                                    pexp,
                                    jc,
                                )
                                if jc == 1 and pending_norm is not None:
                                    emit_norm(pending_norm)
                                    pending_norm = None
                                elif deferred:
                                    deferred.pop(0)()
                            v_st_p, pexp_p, jc_p = prev_pv
                            for k in range(IB // 512):
                                nc.tensor.matmul(
                                    acc[:, 512 * k : 512 * (k + 1)],
                                    v_st_p,
                                    pexp_p[:, 512 * k : 512 * (k + 1)],
                                    start=False,
                                    stop=True,
                                )
                            pending_norm = (acc, r, qs, isl_)
                    while deferred:
                        deferred.pop(0)()
                    if flush_norm:
                        emit_norm(pending_norm)
                        pending_norm = None
                    pending_norm_box[0] = pending_norm

                def emit_k_dmas(sblk):
                    kx = []
                    for d in range(DCH):
                        t = kx_pool.tile([128, 512], BF16, tag="kx", name="kx")
                        nc.sync.dma_start(
                            out=t[:],
                            in_=kt_d[128 * d : 128 * (d + 1), 512 * sblk : 512 * (sblk + 1)],
                        )
                        kx.append(t)
                    return kx

                NSB = S // 512
                with nc.named_scope("chase"):
                    acc0 = ps_acc.tile([65, IB], F32, tag="acc", name="acc0")
                    prev_pv = None
                    kx_next = emit_k_dmas(0)
                    emit_kproj_pair(0, 0, kx_next)
                    emit_kproj_pair(0, 1, kx_next)
                    for sblk in range(NSB):
                        if sblk + 1 < NSB:
                            kx_next = emit_k_dmas(sblk + 1)
                        for jc in range(4 * sblk, 4 * sblk + 4):
                            s_ps = ps_sc.tile([128, IB], F32, tag="sc", name="s_ps")
                            for k in range(IB // 512):
                                nc.tensor.matmul(
                                    s_ps[:, 512 * k : 512 * (k + 1)],
                                    khT[0][0:64, 128 * jc : 128 * (jc + 1)],
                                    qhT[0][0:64, 512 * k : 512 * (k + 1)],
                                    start=True,
                                    stop=True,
                                )
                            if prev_pv is not None:
                                v_st_p, pexp_p, jc_p = prev_pv
                                for k in range(IB // 512):
                                    nc.tensor.matmul(
                                        acc0[:, 512 * k : 512 * (k + 1)],
                                        v_st_p,
                                        pexp_p[:, 512 * k : 512 * (k + 1)],
                                        start=(jc_p == 0),
                                        stop=False,
                                    )
                            pexp = pexp_pool.tile([128, IB], BF16, tag="pexp", name="pexp")
                            nc.scalar.activation(pexp[:], s_ps[:], AF.Exp, scale=INV_SQRT_DK)
                            prev_pv = (vhe[0][:, 130 * jc : 130 * jc + 65], pexp, jc)
                        if sblk + 1 < NSB:
                            emit_kproj_pair(sblk + 1, 0, kx_next)
                            emit_kproj_pair(sblk + 1, 1, kx_next)
                    v_st_p, pexp_p, jc_p = prev_pv
                    for k in range(IB // 512):
                        nc.tensor.matmul(
                            acc0[:, 512 * k : 512 * (k + 1)],
                            v_st_p,
                            pexp_p[:, 512 * k : 512 * (k + 1)],
                            start=False,
                            stop=True,
                        )
                    pending_norm_box[0] = (acc0, 0, slice(0, 64), slice(0, IB))

                # khT complete: quantize the fp8 stationaries in background
                # (r=1 first — heads 2,3 run soonest on the fp8 path)
                emit_quantK(1)
                emit_quantK(0)

                nc.sync.dma_start(
                    out=wf_sb[:].rearrange("p (c f) -> p c f", c=2),
                    in_=wf_d.rearrange("(c p) f -> p c f", p=128),
                )
                qx1 = emit_q_dmas(1)
                emit_attention(0, [], heads=[1, 2], flush_norm=False, fp8=(2,))
                emit_attention(0, make_qproj_closures(1, 0, qx1), heads=[3], fp8=(3,))
                final0 = [
                    (lambda f=f, i4=i4: emit_final_tile(0, f, i4))
                    for f in range(D // 128)
                    for i4 in range(IB // 512)
                ]
                emit_attention(
                    1, make_qproj_closures(1, 1, qx1), heads=[0, 1],
                    flush_norm=False, fp8=(0, 1),
                )
                emit_attention(1, final0, heads=[2, 3], flush_norm=False, fp8=(2, 3))
                acc_, r_, qs_, isl_ = pending_norm_box[0]
                pending_norm_box[0] = None
                with nc.named_scope("final1"):
                    for k in range(IB // 512):
                        i0 = IB + 512 * k
                        rc = rc_pool.tile([1, 512], BF16, tag="rc", name="rc")
                        nc.vector.reciprocal(rc[:], acc_[64:65, 512 * k : 512 * (k + 1)])
                        bc_ps = ps_sc.tile([128, 512], F32, tag="sc", name="bc_ps")
                        nc.tensor.matmul(
                            bc_ps[0:64, :], ones128[0:1, 0:64], rc[:], start=True, stop=True
                        )
                        bc_sb = bc_pool.tile([64, 512], F32, tag="bc", name="bc_sb")
                        nc.vector.tensor_copy(bc_sb[:], bc_ps[0:64, :])
                        nc.vector.tensor_tensor(
                            outT[r_][qs_, i0 : i0 + 512],
                            acc_[0:64, 512 * k : 512 * (k + 1)],
                            bc_sb[:],
                            mybir.AluOpType.mult,
                        )
                        for n, f in enumerate(range(D // 128)):
                            emit_final_tile(
                                1, f, k,
                                pool=(ps_acc if n % 2 else None),
                                copy_eng=("act" if n % 2 else None),
                            )

    nc.compile()
    return nc


def _get_nc():
    if "nc" not in _CACHE:
        _CACHE["nc"] = _build()
    return _CACHE["nc"]


def _bf(x):
    return np.ascontiguousarray(np.asarray(x, dtype=np.float32)).astype(BFNP)


def kernel(Q, K, V, Wq, bq, Wk, bk, Wv, bv, Wf, bf):
    Q, K, V = np.asarray(Q), np.asarray(K), np.asarray(V)
    Wq, Wk, Wv, Wf = (np.asarray(a) for a in (Wq, Wk, Wv, Wf))
    bq, bk, bv, bf = (np.asarray(a) for a in (bq, bk, bv, bf))

    nc = _get_nc()

    qt = [_bf(Q[b].T) for b in range(B)]
    kt = [_bf(K[b].T) for b in range(B)]
    vt = [_bf(V[b].T) for b in range(B)]
    wq_g = [_bf(Wq[HPG * g : HPG * (g + 1)].transpose(1, 0, 2).reshape(D, CW)) for g in range(GPB)]
    wk_g = [_bf(Wk[HPG * g : HPG * (g + 1)].transpose(1, 0, 2).reshape(D, CW)) for g in range(GPB)]
    wv_g = [_bf(Wv[HPG * g : HPG * (g + 1)].transpose(1, 0, 2).reshape(D, CW)) for g in range(GPB)]
    wf_g = [_bf(Wf[CW * g : CW * (g + 1), :]) for g in range(GPB)]
    bq_g = [np.ascontiguousarray(bq[HPG * g : HPG * (g + 1)].reshape(CW), np.float32) for g in range(GPB)]
    bk_g = [np.ascontiguousarray(bk[HPG * g : HPG * (g + 1)].reshape(CW), np.float32) for g in range(GPB)]
    bv_g = [_bf(bv[HPG * g : HPG * (g + 1)].reshape(1, CW)) for g in range(GPB)]

    ones_col = np.ones((128, 2 * JCH, 1), BFNP)
    ones_row = np.ones((1, 128), BFNP)
    in_maps = []
    for c in range(NCORES):
        b, g = c // GPB, c % GPB
        in_maps.append(
            {
                "qt": qt[b], "kt": kt[b], "vt": vt[b],
                "wq": wq_g[g], "wk": wk_g[g], "wv": wv_g[g], "wf": wf_g[g],
                "bq": bq_g[g], "bk": bk_g[g], "bv": bv_g[g],
                "ones32": ones_col, "ones_row": ones_row,
            }
        )

    res = run_bass_kernel_spmd(nc, in_maps, list(range(NCORES)))

    out = np.empty((B, S, D), np.float32)
    bf32 = bf.astype(np.float32)
    for b in range(B):
        acc = res.results[GPB * b]["pt"].astype(np.float32)
        for g in range(1, GPB):
            acc = acc + res.results[GPB * b + g]["pt"].astype(np.float32)
        out[b] = acc.T + bf32
    return out
